# revision 29
# baseline (speedup 1.0000x reference)
"""Trainium2 Bass kernel for a 12-layer autoregressive transformer.

Sharding: 4 batch elements x 2-way sequence split across 8 cores.
Core pair p = (2p, 2p+1) handles batch element p. Within a pair, core
half 0 owns 128-token blocks [0,3,4,7], half 1 owns [1,2,5,6] (this
balances causal-attention work exactly: 18 block-pairs each).

Cross-core exchange per layer: the bf16 LN1 output (x-hat) is traded
within each pair via a ReduceScatter(add) over a buffer whose "self"
chunk is pre-zeroed — each core receives exactly its peer's x-hat at
half the AllGather cost — and each core recomputes the peer K/V
locally (cheap: matmul cost on the PE scales only with output columns).
The exchange launches right after LN1 so QKV + local attention overlap
it; attention then runs local key blocks first, remote blocks after
the collective lands.

LayerNorm uses column-centered weights (mean subtraction folded into
the weights) and variance via ones-column matmuls on the PE. On-device
layout is feature-major; weights and moving operands are bf16 on the
PE (full rate); the residual stream stays f32r; attention softmax
denominators come from a ones-column appended to V.
"""

import os
import numpy as np
import ml_dtypes

import concourse.bass as bass
import concourse.mybir as mybir
import concourse.tile as tile
from concourse import bacc
from concourse.bass_utils import run_bass_kernel_spmd

F32 = mybir.dt.float32
F32R = mybir.dt.float32r
BF16 = mybir.dt.bfloat16

S, D, H, HD, L, DFF, VOCAB = 1024, 512, 8, 64, 12, 2048, 19
SCHEMA, NDIMS = 21, 64
NB, TB = 8, 128            # token blocks of 128
TLOC = 512                 # tokens per core
DC = D // 128              # 4 feature chunks
H0_BLOCKS = [0, 3, 4, 7]
H1_BLOCKS = [1, 2, 5, 6]
# padded q-window widths per key block (max over the two halves' suffix counts)
# virtual attention slots: 4 local blocks then 4 remote (peer) blocks, each
# ordered ascending; q-window width for slot s is (4 - s) * 128 padded to the
# max over halves -- identical for both halves by construction of the split.
NPAD_V = [512, 384, 256, 128, 512, 384, 256, 128]
OFF = np.concatenate([[0], np.cumsum(NPAD_V)]).astype(int)
SUM_NPAD = int(OFF[-1])                # 2816

XB_ELEMS = DC * 128 * TLOC             # bf16 elems of one x-hat half (262144)
XB_WORDS = XB_ELEMS // 2               # f32 words (131072)
PAIRS = [[0, 1], [2, 3], [4, 5], [6, 7]]

_PROGRAM_CACHE = {}
LAST_RESULTS = None
LAST_EXEC_S = None


def _run_spmd(nc, in_maps, n_cores=8, bench_reps=0):
    """Execute a prebuilt Bass module on 8 cores via PJRT (axon), jitting
    once. Benchmarking uses pipelined dispatch: N back-to-back executes
    amortize the ~60 ms tunnel round-trip latency, so the marginal
    per-call time (slope between two pipeline depths) estimates the true
    per-invocation hardware execution time."""
    global LAST_EXEC_S
    import time
    import jax
    from jax.experimental.shard_map import shard_map
    from jax.sharding import Mesh, PartitionSpec, NamedSharding
    from concourse import bass2jax, mybir as _mybir
    bass2jax.install_neuronx_cc_hook()

    partition_name = nc.partition_id_tensor.name if nc.partition_id_tensor else None
    in_names, out_names, out_avals, zero_outs = [], [], [], []
    for alloc in nc.m.functions[0].allocations:
        if not isinstance(alloc, _mybir.MemoryLocationSet):
            continue
        name = alloc.memorylocations[0].name
        if alloc.kind == "ExternalInput":
            if name != partition_name:
                in_names.append(name)
        elif alloc.kind == "ExternalOutput":
            shape = tuple(alloc.tensor_shape)
            dtype = _mybir.dt.np(alloc.dtype)
            out_names.append(name)
            out_avals.append(jax.core.ShapedArray(shape, dtype))
            zero_outs.append(np.zeros(shape, dtype))
    n_params = len(in_names)
    n_outs = len(out_avals)
    all_in_names = list(in_names) + list(out_names)
    if partition_name is not None:
        all_in_names.append(partition_name)

    def _body(*args):
        operands = list(args)
        if partition_name is not None:
            operands.append(bass2jax.partition_id_tensor())
        outs = bass2jax._bass_exec_p.bind(
            *operands, out_avals=tuple(out_avals), in_names=tuple(all_in_names),
            out_names=tuple(out_names), lowering_input_output_aliases=(),
            sim_require_finite=True, sim_require_nnan=True, nc=nc)
        return tuple(outs)

    devices = jax.devices()[:n_cores]
    mesh = Mesh(np.asarray(devices), ("core",))
    in_specs = (PartitionSpec("core"),) * (n_params + n_outs)
    out_specs = (PartitionSpec("core"),) * n_outs
    donate = tuple(range(n_params, n_params + n_outs))
    sharded = jax.jit(
        shard_map(_body, mesh=mesh, in_specs=in_specs, out_specs=out_specs,
                  check_rep=False),
        donate_argnums=donate, keep_unused=True)

    concat_in = [np.concatenate([np.asarray(in_maps[c][nm])[None]
                                 for c in range(n_cores)], axis=0)
                 .reshape(n_cores * np.asarray(in_maps[0][nm]).shape[0],
                          *np.asarray(in_maps[0][nm]).shape[1:])
                 for nm in in_names]
    sh = NamedSharding(mesh, PartitionSpec("core"))

    def _zeros_dev():
        zo = [jax.device_put(np.zeros((n_cores * z.shape[0], *z.shape[1:]),
                                      z.dtype), sh) for z in zero_outs]
        jax.block_until_ready(zo)
        return zo

    dev_in = [jax.device_put(a, sh) for a in concat_in]
    jax.block_until_ready(dev_in)
    out_arrs = jax.block_until_ready(sharded(*dev_in, *_zeros_dev()))

    if bench_reps:
        def pipeline_time(n):
            zos = [_zeros_dev() for _ in range(n)]
            t0 = time.perf_counter()
            rs = [sharded(*dev_in, *zo) for zo in zos]
            jax.block_until_ready(rs)
            return time.perf_counter() - t0
        n1, n2 = 6, 26
        pipeline_time(2)   # warm the pipelined path
        t1s, t2s = [], []
        for _ in range(max(5, 2 * bench_reps)):
            t1s.append(pipeline_time(n1))
            t2s.append(pipeline_time(n2))
        # min of each endpoint separately: ambient spikes in either batch
        # can only inflate a sample, never deflate the quiet-window value
        LAST_EXEC_S = (min(t2s) - min(t1s)) / (n2 - n1)

    return [{nm: np.asarray(out_arrs[i]).reshape(n_cores, *out_avals[i].shape)[c]
             for i, nm in enumerate(out_names)} for c in range(n_cores)]


def _build_program(n_layers=L, bias_flags=(False, False, False, False, False)):
    nc = bacc.Bacc("TRN2", target_bir_lowering=False, num_devices=8)

    # ---------------- DRAM I/O ----------------
    xsT_d = nc.dram_tensor("xsT", [NDIMS, TLOC], F32R, kind="ExternalInput")
    posT_d = nc.dram_tensor("posT", [DC, 128, TLOC], F32, kind="ExternalInput")
    masks_d = nc.dram_tensor("masks", [2, 128, SUM_NPAD], BF16, kind="ExternalInput")
    rw_d = nc.dram_tensor("rw", [NDIMS, D], F32R, kind="ExternalInput")
    w1_d = nc.dram_tensor("w1", [L, D, 3 * D], BF16, kind="ExternalInput")
    wp_d = nc.dram_tensor("wp", [L, D, D], BF16, kind="ExternalInput")
    w2_d = nc.dram_tensor("w2", [L, D, DFF], BF16, kind="ExternalInput")
    w3_d = nc.dram_tensor("w3", [L, DFF, D], BF16, kind="ExternalInput")
    wo_d = nc.dram_tensor("wo", [D, VOCAB], BF16, kind="ExternalInput")
    b1_d = nc.dram_tensor("b1", [L, 3 * D], F32R, kind="ExternalInput")
    bp_d = nc.dram_tensor("bp", [L, D], F32R, kind="ExternalInput")
    b2_d = nc.dram_tensor("b2", [L, 128, DFF // 128], F32, kind="ExternalInput")
    b3_d = nc.dram_tensor("b3", [L, D], F32R, kind="ExternalInput")
    bo_d = nc.dram_tensor("bo", [VOCAB], F32R, kind="ExternalInput")
    out_d = nc.dram_tensor("outT", [VOCAB, TLOC], F32, kind="ExternalOutput")

    bounce = nc.dram_tensor("bounce", [2 * XB_WORDS], F32)
    agout = nc.dram_tensor("agout", [XB_WORDS], F32)

    with tile.TileContext(nc) as tc:
        _emit(nc, tc, locals(), n_layers, bias_flags)
    nc.compile()
    return nc


def _emit(nc, tc, d, n_layers, bias_flags):
    add_b1, add_bp, add_b2, add_b3, add_bo = bias_flags
    xsT_d, posT_d, masks_d, rw_d = d["xsT_d"], d["posT_d"], d["masks_d"], d["rw_d"]
    w1_d, wp_d, w2_d, w3_d, wo_d = d["w1_d"], d["wp_d"], d["w2_d"], d["w3_d"], d["wo_d"]
    b1_d, bp_d, b2_d, b3_d, bo_d = d["b1_d"], d["bp_d"], d["b2_d"], d["b3_d"], d["bo_d"]
    out_d, bounce, agout = d["out_d"], d["bounce"], d["agout"]
    AF = mybir.ActivationFunctionType

    import contextlib
    ctx = contextlib.ExitStack()
    persist = ctx.enter_context(tc.tile_pool(name="persist", bufs=1))
    scr = ctx.enter_context(tc.tile_pool(name="scr", bufs=1))
    wpool = ctx.enter_context(tc.tile_pool(name="wpool", bufs=10))
    ppool = ctx.enter_context(tc.tile_pool(name="ppool", bufs=6))
    small = ctx.enter_context(tc.tile_pool(name="small", bufs=4))
    recp = ctx.enter_context(tc.tile_pool(name="recp", bufs=1))
    ps_mm = ctx.enter_context(tc.tile_pool(name="ps_mm", bufs=3, space="PSUM"))
    ps_bc = ctx.enter_context(tc.tile_pool(name="ps_bc", bufs=2, space="PSUM"))
    ps_ctx = ctx.enter_context(tc.tile_pool(name="ps_ctx", bufs=3, space="PSUM"))

    # ---- persistent tiles ----
    h = persist.tile([128, DC, TLOC], F32R)
    x2 = persist.tile([128, DC, TLOC], F32R)
    xb = persist.tile([128, DC, TLOC], BF16)      # bf16 x-hat (own tokens)
    xrem = persist.tile([128, DC, TLOC], BF16)    # bf16 x-hat (peer tokens)
    qT = persist.tile([128, DC, TLOC], BF16)
    kst = persist.tile([128, DC, TLOC], BF16)     # local k, feature-major
    kst2 = persist.tile([128, DC, TLOC], BF16)    # peer k, feature-major
    vloc = persist.tile([128, 4, H, HD + 1], BF16)
    vrem = persist.tile([128, 4, H, HD + 1], BF16)
    mt = persist.tile([128, 2, SUM_NPAD], BF16)
    ctxf = persist.tile([128, DC, TLOC], BF16)
    ctxl = persist.tile([HD + 1, H, TLOC], F32)   # local-pass ctx+denominator
    gel = persist.tile([128, 16, TLOC], BF16)
    onesc = persist.tile([128, 1], F32R)        # 1/512 column (mean via matmul)
    onesr = persist.tile([1, TLOC], F32R)       # exact ones row
    epst = persist.tile([1, 1], F32R)
    xsT = persist.tile([NDIMS, TLOC], F32R)
    rw = persist.tile([NDIMS, D], F32R)

    nc.vector.memset(onesc[:].bitcast(F32), 1.0 / D)
    nc.vector.memset(onesr[:].bitcast(F32), 1.0)
    nc.vector.memset(epst[:].bitcast(F32), 1e-5)
    # gel doubles as the zero source for the one-time bounce clear
    nc.gpsimd.memset(gel[:, 0:DC, :].rearrange("p c t -> p (c t)"), 0.0)
    nc.gpsimd.memset(vloc[:, :, :, HD:HD + 1], 1.0)
    nc.gpsimd.memset(vrem[:, :, :, HD:HD + 1], 1.0)
    nc.sync.dma_start(out=mt[:], in_=masks_d.rearrange("t p n -> p t n"))
    nc.sync.dma_start(out=xsT[:], in_=xsT_d[:])
    nc.sync.dma_start(out=rw[:], in_=rw_d[:])
    bot = None
    if add_bo:
        bot = persist.tile([1, VOCAB], F32R)
        nc.sync.dma_start(out=bot[:], in_=bo_d[None, :])

    # pair parity -> dynamic write base into bounce (peer's RS chunk)
    eng = nc.gpsimd
    pid = eng.partition_id()
    rpar = eng.alloc_register("rpar")
    eng.reg_mod(rpar, pid, 2)
    rpeer = eng.alloc_register("rpeer")
    eng.reg_alu(rpeer, 1, rpar, mybir.AluOpType.subtract)
    rxb = eng.alloc_register("rxb")
    eng.reg_mul(rxb, rpeer, XB_ELEMS)
    xbase_sv = eng.snap(rxb, donate=True, min_val=0, max_val=XB_ELEMS)

    # zero both RS chunks once; the self chunk stays zero for all layers
    for half in range(2):
        nc.sync.dma_start(
            out=bounce[:].bitcast(BF16)[half * XB_ELEMS:(half + 1) * XB_ELEMS]
                .rearrange("(p t) -> p t", p=128),
            in_=gel[:, 0:DC, :].rearrange("p c t -> p (c t)"))

    # ---- embed: h = read_w.T @ xsT + posT ----
    posTt = scr.tile([128, DC, TLOC], F32, tag="s8b")
    nc.sync.dma_start(out=posTt[:], in_=posT_d.rearrange("c p t -> p c t"))
    for oc in range(DC):
        ps = ps_mm.tile([128, TLOC], F32, tag="mm")
        nc.tensor.matmul(ps[:], rw[0:NDIMS, oc * 128:(oc + 1) * 128], xsT[:],
                         start=True, stop=True)
        with nc.allow_low_precision(reason="f32r residual stream"):
            nc.vector.tensor_add(h[:, oc, :], ps[:], posTt[:, oc, :])

    def layernorm(dst):
        """h (f32) -> dst (bf16) = (h * rstd); mean handled by centered
        weights downstream. Per-chunk emission so downstream matmuls can
        start as soon as their chunk is scaled."""
        with nc.allow_low_precision(reason="f32r x^2 for LN stats"):
            for c in range(DC):
                nc.gpsimd.tensor_mul(x2[:, c, :], h[:, c, :], h[:, c, :])
        mu_ps = ps_bc.tile([1, TLOC], F32, tag="bc")
        e2_ps = ps_bc.tile([1, TLOC], F32, tag="bc")
        for c in range(DC):
            nc.tensor.matmul(mu_ps[:], onesc[:], h[:, c, :],
                             start=(c == 0), stop=(c == DC - 1))
        for c in range(DC):   # e2 = sum(x^2)/D + eps (eps via rank-1 matmul)
            nc.tensor.matmul(e2_ps[:], onesc[:], x2[:, c, :],
                             start=(c == 0), stop=False)
        nc.tensor.matmul(e2_ps[:], epst[:], onesr[:],
                         start=False, stop=True)
        musq = small.tile([1, TLOC], F32, tag="sm")
        var = small.tile([1, TLOC], F32, tag="sm")
        rr = small.tile([1, TLOC], F32R, tag="sm")
        nc.scalar.square(out=musq[:], in_=mu_ps[:])
        nc.vector.tensor_sub(var[:], e2_ps[:], musq[:])
        nc.scalar.activation(out=var[:], in_=var[:], func=AF.Sqrt)
        with nc.allow_low_precision(reason="f32r rstd is plenty for LN"):
            nc.vector.reciprocal(rr[:], var[:])
        rb_ps = ps_bc.tile([128, TLOC], F32, tag="bc")
        nc.tensor.matmul(rb_ps[:], onesr[0:1, 0:128], rr[:], start=True, stop=True)
        for c in range(DC):
            nc.vector.tensor_mul(dst[:, c, :], h[:, c, :], rb_ps[:])

    for li in range(n_layers):
        i = li % L
        lt = 0 if i < 2 else 1
        w1k = wpool.tile([128, DC, D], BF16, tag="w")
        nc.sync.dma_start(out=w1k[:], in_=w1_d[i, :, D:2 * D]
                          .rearrange("(c p) o -> p c o", p=128))
        w1v = wpool.tile([128, DC, D], BF16, tag="w")
        nc.sync.dma_start(out=w1v[:], in_=w1_d[i, :, 2 * D:3 * D]
                          .rearrange("(c p) o -> p c o", p=128))
        w1q = wpool.tile([128, DC, D], BF16, tag="w")
        nc.sync.dma_start(out=w1q[:], in_=w1_d[i, :, 0:D]
                          .rearrange("(c p) o -> p c o", p=128))
        wpt = wpool.tile([128, DC, D], BF16, tag="w")
        nc.sync.dma_start(out=wpt[:], in_=wp_d[i].rearrange("(c p) o -> p c o", p=128))
        if add_b1:
            b1t = small.tile([1, 3 * D], F32R, tag="bias")
            nc.sync.dma_start(out=b1t[:], in_=b1_d[i][None, :])

        # ---- LN1 -> xb (bf16), exchange xb within the pair ----
        with nc.named_scope(f"ln1_{li}"):
            layernorm(xb)
        nc.gpsimd.dma_start(
            out=bounce[:].bitcast(BF16)[bass.ds(xbase_sv, XB_ELEMS)]
                .rearrange("(p n) -> p n", p=128),
            in_=xb[:].rearrange("p c t -> p (c t)"))
        nc.gpsimd.collective_compute(
            "ReduceScatter", mybir.AluOpType.add, replica_groups=PAIRS,
            ins=[bounce[:]], outs=[agout[:]])

        # ---- local k, v, q (overlap the collective) ----
        for oc in range(DC):   # k, feature-major
            ps = ps_mm.tile([128, TLOC], F32, tag="mm")
            for c in range(DC):
                nc.tensor.matmul(ps[:], w1k[:, c, oc * 128:(oc + 1) * 128],
                                 xb[:, c, :], start=(c == 0),
                                 stop=(c == DC - 1 and not add_b1))
            if add_b1:
                nc.tensor.matmul(ps[:], b1t[0:1, (DC + oc) * 128:(DC + oc + 1) * 128],
                                 onesr[:], start=False, stop=True)
            nc.scalar.copy(out=kst[:, oc, :], in_=ps[:])
        # v token-major: vT = xb.T @ Wv  (x stationary, W moving)
        for tcb in range(4):
            ps = ps_mm.tile([128, TLOC], F32, tag="mm")
            for c in range(DC):
                nc.tensor.matmul(ps[:], xb[:, c, tcb * 128:(tcb + 1) * 128],
                                 w1v[:, c, :], start=(c == 0),
                                 stop=(c == DC - 1 and not add_b1))
            if add_b1:
                nc.tensor.matmul(ps[:], onesr[0:1, 0:128],
                                 b1t[0:1, 2 * D:3 * D], start=False, stop=True)
            nc.vector.tensor_copy(
                vloc[:, tcb, :, 0:HD], ps[:].rearrange("p (h d) -> p h d", h=H))
        for oc in range(DC):   # q
            ps = ps_mm.tile([128, TLOC], F32, tag="mm")
            for c in range(DC):
                nc.tensor.matmul(ps[:], w1q[:, c, oc * 128:(oc + 1) * 128],
                                 xb[:, c, :], start=(c == 0),
                                 stop=(c == DC - 1 and not add_b1))
            if add_b1:
                nc.tensor.matmul(ps[:], b1t[0:1, oc * 128:(oc + 1) * 128],
                                 onesr[:], start=False, stop=True)
            nc.scalar.copy(out=qT[:, oc, :], in_=ps[:])

        # ---- attention: score/exp/mask, then PV (emitted separately so
        # two heads can software-pipeline: head B's score matmul fills the
        # PE gap while head A's exp/mask chain runs) ----
        def attn_score(hh, vi):
            hc, hr = hh // 2, (hh % 2) * HD
            remote, s = vi >= 4, vi % 4
            w = NPAD_V[vi]
            ksrc = kst2 if remote else kst
            klhs = ksrc[hr:hr + HD, hc, s * 128:(s + 1) * 128]
            sps = ps_mm.tile([128, TLOC], F32, tag="mm")
            nc.tensor.matmul(sps[:, 0:w], klhs,
                             qT[hr:hr + HD, hc, TLOC - w:TLOC],
                             start=True, stop=True)
            pt = ppool.tile([128, TLOC], BF16, tag="P")
            nc.scalar.activation(out=pt[:, 0:w], in_=sps[:, 0:w],
                                 func=AF.Exp, scale=0.125)
            mw = w if (lt == 0 and s == 0) else 128
            nc.vector.tensor_mul(pt[:, 0:mw], pt[:, 0:mw],
                                 mt[:, lt, OFF[vi]:OFF[vi] + mw])
            return pt

        def attn_pv(hh, vi, pt, cps):
            remote, s = vi >= 4, vi % 4
            w = NPAD_V[vi]
            vlhs = vrem[:, s, hh, :] if remote else vloc[:, s, hh, :]
            nc.tensor.matmul(cps[:, TLOC - w:TLOC], vlhs,
                             pt[:, 0:w], start=(vi % 4 == 0), stop=(vi % 4 == 3))

        # ---- local attention pass (overlaps the collective): all heads'
        # local slots accumulate in PSUM, staged to SBUF ----
        for hh in range(H):
            cps = ps_ctx.tile([HD + 1, TLOC], F32, tag="ctx")
            for vi in range(4):
                attn_pv(hh, vi, attn_score(hh, vi), cps)
            nc.vector.tensor_copy(ctxl[:, hh, :], cps[:])

        # ---- peer x-hat arrives; recompute peer k, v ----
        for c in range(DC):
            nc.sync.dma_start(
                out=xrem[:, c, :],
                in_=agout[:].bitcast(BF16)[0:XB_ELEMS]
                    .rearrange("(p n) -> p n", p=128)
                    [:, c * TLOC:(c + 1) * TLOC])
        for oc in range(DC):   # peer k
            ps = ps_mm.tile([128, TLOC], F32, tag="mm")
            for c in range(DC):
                nc.tensor.matmul(ps[:], w1k[:, c, oc * 128:(oc + 1) * 128],
                                 xrem[:, c, :], start=(c == 0),
                                 stop=(c == DC - 1 and not add_b1))
            if add_b1:
                nc.tensor.matmul(ps[:], b1t[0:1, (DC + oc) * 128:(DC + oc + 1) * 128],
                                 onesr[:], start=False, stop=True)
            nc.scalar.copy(out=kst2[:, oc, :], in_=ps[:])
        for tcb in range(4):   # peer v
            ps = ps_mm.tile([128, TLOC], F32, tag="mm")
            for c in range(DC):
                nc.tensor.matmul(ps[:], xrem[:, c, tcb * 128:(tcb + 1) * 128],
                                 w1v[:, c, :], start=(c == 0),
                                 stop=(c == DC - 1 and not add_b1))
            if add_b1:
                nc.tensor.matmul(ps[:], onesr[0:1, 0:128],
                                 b1t[0:1, 2 * D:3 * D], start=False, stop=True)
            nc.vector.tensor_copy(
                vrem[:, tcb, :, 0:HD], ps[:].rearrange("p (h d) -> p h d", h=H))

        # ---- remote attention pass + finalize ----
        for hh in range(H):
            cps2 = ps_ctx.tile([HD + 1, TLOC], F32, tag="ctx")
            for vi in range(4, NB):
                attn_pv(hh, vi, attn_score(hh, vi), cps2)
            nc.vector.tensor_add(ctxl[:, hh, :], ctxl[:, hh, :], cps2[:])
        rec = recp.tile([1, H, TLOC], F32R, tag="rec")
        with nc.allow_low_precision(reason="f32r softmax denom recip"):
            nc.vector.reciprocal(rec[:].rearrange("p h t -> p (h t)"),
                                 ctxl[HD:HD + 1, :, :].rearrange("p h t -> p (h t)"))
        for hh in range(H):
            hc, hr = hh // 2, (hh % 2) * HD
            rb2 = ps_bc.tile([HD, TLOC], F32, tag="bc", name=f"rb{hh}")
            nc.tensor.matmul(rb2[:], onesr[0:1, 0:HD], rec[:, hh, :],
                             start=True, stop=True)
            nc.vector.tensor_mul(ctxf[hr:hr + HD, hc, :], ctxl[0:HD, hh, :], rb2[:])

        # ---- attention out-projection + residual ----
        if add_bp:
            bpt = small.tile([1, D], F32R, tag="bias")
            nc.sync.dma_start(out=bpt[:], in_=bp_d[i][None, :])
        for oc in range(DC):
            ps = ps_mm.tile([128, TLOC], F32, tag="mm")
            for c in range(DC):
                nc.tensor.matmul(ps[:], wpt[:, c, oc * 128:(oc + 1) * 128],
                                 ctxf[:, c, :], start=(c == 0),
                                 stop=(c == DC - 1 and not add_bp))
            if add_bp:
                nc.tensor.matmul(ps[:], bpt[0:1, oc * 128:(oc + 1) * 128],
                                 onesr[:], start=False, stop=True)
            with nc.allow_low_precision(reason="f32r residual stream"):
                nc.vector.tensor_add(h[:, oc, :], h[:, oc, :], ps[:])

        # ---- LN2 + MLP ----
        w2q = []
        for qi in range(4):
            w2t = wpool.tile([128, DC, D], BF16, tag="w", name=f"w2_{qi}")
            nc.sync.dma_start(out=w2t[:], in_=w2_d[i, :, qi * D:(qi + 1) * D]
                              .rearrange("(c p) o -> p c o", p=128))
            w2q.append(w2t)
        layernorm(xb)
        w3q = []
        for qi in range(4):
            w3t = wpool.tile([128, DC, D], BF16, tag="w", name=f"w3_{qi}")
            nc.sync.dma_start(out=w3t[:], in_=w3_d[i, qi * D:(qi + 1) * D, :]
                              .rearrange("(c p) o -> p c o", p=128))
            w3q.append(w3t)
        if add_b2:
            b2t = small.tile([128, DFF // 128], F32, tag="bias")
            nc.sync.dma_start(out=b2t[:], in_=b2_d[i])
        if add_b3:
            b3t = small.tile([1, D], F32R, tag="bias")
            nc.sync.dma_start(out=b3t[:], in_=b3_d[i][None, :])
        for oc in range(16):
            ps = ps_mm.tile([128, TLOC], F32, tag="mm")
            for c in range(DC):
                nc.tensor.matmul(ps[:], w2q[oc // 4][:, c, (oc % 4) * 128:(oc % 4 + 1) * 128],
                                 xb[:, c, :], start=(c == 0),
                                 stop=(c == DC - 1))
            bias_arg = b2t[:, oc:oc + 1] if add_b2 else 0.0
            nc.scalar.activation(out=gel[:, oc, :], in_=ps[:],
                                 func=AF.Gelu_apprx_tanh, bias=bias_arg)
        for oc in range(DC):
            pp = ps_mm.tile([128, TLOC], F32, tag="mm")
            for kc in range(16):
                nc.tensor.matmul(pp[:], w3q[kc // 4][:, kc % 4, oc * 128:(oc + 1) * 128],
                                 gel[:, kc, :], start=(kc == 0),
                                 stop=(kc == 15 and not add_b3))
            if add_b3:
                nc.tensor.matmul(pp[:], b3t[0:1, oc * 128:(oc + 1) * 128],
                                 onesr[:], start=False, stop=True)
            with nc.allow_low_precision(reason="f32r residual stream"):
                nc.vector.tensor_add(h[:, oc, :], h[:, oc, :], pp[:])

    # ---- final LN + vocab projection ----
    wot = persist.tile([128, DC, VOCAB], BF16)
    nc.sync.dma_start(out=wot[:], in_=wo_d.rearrange("(c p) v -> p c v", p=128))
    layernorm(xb)
    ps = ps_mm.tile([VOCAB, TLOC], F32, tag="mm")
    for c in range(DC):
        nc.tensor.matmul(ps[:], wot[:, c, :], xb[:, c, :],
                         start=(c == 0), stop=(c == DC - 1 and not add_bo))
    if add_bo:
        nc.tensor.matmul(ps[:], bot[:], onesr[:], start=False, stop=True)
    osb = small.tile([VOCAB, TLOC], F32, tag="osb")
    nc.scalar.copy(out=osb[:], in_=ps[:])
    nc.sync.dma_start(out=out_d[:], in_=osb[:])
    ctx.close()


def _valid_full():
    """valid[lt, k, q] over global token ids."""
    q = np.arange(S)[None, :]
    k = np.arange(S)[:, None]
    causal = k <= q
    # layer type 0 (mask_first)
    schema_q = q < SCHEMA
    blk = (k // 4 == q // 4) & (q < 20) & (k < 20)
    row20 = (q == 20) & (k <= 20)
    path0 = (q >= SCHEMA) & (k >= SCHEMA)
    m0 = (blk | row20 | path0) & causal
    return np.stack([m0, causal])


def _prep(inputs):
    f32 = lambda a: np.ascontiguousarray(np.asarray(a), dtype=np.float32)
    xs = f32(inputs["xs"])
    read_w, read_b = f32(inputs["read_w"]), f32(inputs["read_b"])
    pos = np.concatenate([f32(inputs["pos_schema"]),
                          f32(inputs["pos_path"])[: S - SCHEMA]], axis=0)
    ln1_g, ln1_b = f32(inputs["ln1_g"]), f32(inputs["ln1_b"])
    ln2_g, ln2_b = f32(inputs["ln2_g"]), f32(inputs["ln2_b"])
    lnf_g, lnf_b = f32(inputs["lnf_g"]), f32(inputs["lnf_b"])
    attn_w, attn_b = f32(inputs["attn_w"]), f32(inputs["attn_b"])
    attnp_w, attnp_b = f32(inputs["attnp_w"]), f32(inputs["attnp_b"])
    fc_w, fc_b = f32(inputs["fc_w"]), f32(inputs["fc_b"])
    proj_w, proj_b = f32(inputs["proj_w"]), f32(inputs["proj_b"])
    out_w, out_b = f32(inputs["out_w"]), f32(inputs["out_b"])

    w1 = attn_w * ln1_g[:, :, None]
    b1 = np.einsum("ld,ldo->lo", ln1_b, attn_w) + attn_b
    w2 = fc_w * ln2_g[:, :, None]
    b2 = np.einsum("ld,ldo->lo", ln2_b, fc_w) + fc_b
    wo = out_w * lnf_g[:, None]
    bo = lnf_b @ out_w + out_b
    # center the LN-fed weights over the contraction axis: the kernel feeds
    # x*rstd (not (x-mu)*rstd); centered columns make the mu term vanish.
    w1 = w1 - w1.mean(axis=1, keepdims=True)
    w2 = w2 - w2.mean(axis=1, keepdims=True)
    wo = wo - wo.mean(axis=0, keepdims=True)
    b2p = np.ascontiguousarray(
        b2.reshape(L, DFF // 128, 128).transpose(0, 2, 1))

    valid = _valid_full()
    bf = ml_dtypes.bfloat16
    shared = dict(rw=read_w.astype(np.float32),
                  w1=w1.astype(bf), wp=attnp_w.astype(bf),
                  w2=w2.astype(bf), w3=proj_w.astype(bf), wo=wo.astype(bf),
                  b1=b1, bp=attnp_b, b2=b2p, b3=proj_b, bo=bo)

    in_maps = []
    for c in range(8):
        b = c // 2
        blocks = H0_BLOCKS if c % 2 == 0 else H1_BLOCKS
        toks = np.concatenate([np.arange(bb * TB, (bb + 1) * TB) for bb in blocks])
        xsT = np.ascontiguousarray(xs[b][toks].T)                    # (64, 512)
        posT = (pos[toks] + read_b[None, :]).T                        # (512, 512)
        posT = np.ascontiguousarray(posT.reshape(DC, 128, TLOC))
        peer_blocks = H1_BLOCKS if c % 2 == 0 else H0_BLOCKS
        vslot_blocks = list(blocks) + list(peer_blocks)
        masks = np.zeros((2, 128, SUM_NPAD), dtype=bf)
        for lt in range(2):
            for vi, j in enumerate(vslot_blocks):
                w = NPAD_V[vi]
                cols = toks[TLOC - w:]
                masks[lt, :, OFF[vi]:OFF[vi] + w] = \
                    valid[lt, j * TB:(j + 1) * TB][:, cols].astype(bf)
        m = dict(shared)
        m.update(xsT=xsT, posT=posT, masks=masks)
        in_maps.append(m)

    bias_flags = tuple(bool(np.any(v)) for v in
                       (b1, attnp_b, b2, proj_b, bo))
    return in_maps, bias_flags


def kernel(**inputs):
    global LAST_RESULTS
    in_maps, bias_flags = _prep(inputs)
    key = (L, bias_flags)
    if key not in _PROGRAM_CACHE:
        _PROGRAM_CACHE[key] = _build_program(L, bias_flags)
    nc = _PROGRAM_CACHE[key]
    bench = int(os.environ.get("KBENCH_REPS", "0"))
    results = _run_spmd(nc, in_maps, bench_reps=bench)
    LAST_RESULTS = results

    out = np.zeros((4, S, VOCAB), dtype=np.float32)
    for c in range(8):
        b = c // 2
        blocks = H0_BLOCKS if c % 2 == 0 else H1_BLOCKS
        o = results[c]["outT"]                                        # (19, 512)
        for bi, bb in enumerate(blocks):
            out[b, bb * TB:(bb + 1) * TB, :] = o[:, bi * TB:(bi + 1) * TB].T
    return out
